# revision 40
# baseline (speedup 1.0000x reference)
"""Trainium2 Bass kernel for nn_CurvatureOnlyRegularizer (retrieval_knn).

Full inputs -> full output. Shards the 8192 points row-wise across 8 cores.

Per-core pipeline (1024 rows = 8 row-tiles of 128):
  1. S* = e1 . e2^T - n_j via bf16 PE matmul with 2 augmented K-rows carrying
     the -n_j (split hi/lo bf16) term. e1 = bf16(sqrt2*emb), e2 = 2*e1.
  2. ACT evacuates PSUM as t1 = Relu(psum*25 + bias_i) where bias_i folds a
     per-row offset and the 1.5*2^23 magic constant, quantizing -d'^2 to a
     14-bit integer m in fp32's integer binade.
  3. One fused scalar_tensor_tensor pass packs t2 = m + idx_local/512 (iota is
     a preloaded fp16 tile, exact). Per-512-chunk max8 then carries indices
     through selection for free; top-16-of-candidates + max_index on the
     128-wide candidate array recovers (chunk, local) -> global idx.
  4. Neighbor embeddings gathered via dma_gather(transpose=True) directly into
     [D-partition, point*16] layout; PE gram (4 K-chunks) gives all pairwise
     dots incl. the self row/col, so v_l.v_m = G[l,m]-G[l,15]-G[15,m]+G[15,15]
     without ever materializing v.
  5. A 128-DMA "fold" converts block-diagonal G (fp16) to point-major rows.
  6. cos = raw * invd_l * invd_m with d' from the quantized m; upper-triangle
     extracted by ap_gather; signatures sorted descending by max8/match_replace
     rounds and MSE'd against host-reversed reference signatures on ACT.
Host sums the 8 per-core partial sums.
"""

import os
from contextlib import ExitStack

import ml_dtypes
import numpy as np

import concourse.bass as bass
import concourse.bass_isa as bass_isa
import concourse.mybir as mybir
import concourse.tile as tile
from concourse import bacc
from concourse.bass import ds, ts
from concourse.bass_utils import run_bass_kernel_spmd
from concourse.instruction_name_ordered_set import InstructionNameOrderedSet
from concourse.masks import make_identity

N, D, K = 8192, 512, 15
NCORES = 8
SHARD = N // NCORES            # 1024
RT = SHARD // 128              # 8 row-tiles per core
NCH = N // 512                 # 16 column chunks
MAGIC = 12582912.0             # 1.5 * 2^23
C0 = 2052.0
QSCALE = 25.0
PAD_CURV = -1.0
PAD_ANG = -4.0
NEG_BIG = -3.0e38
F32 = mybir.dt.float32
F16 = mybir.dt.float16
BF16 = mybir.dt.bfloat16
I16 = mybir.dt.int16
U32 = mybir.dt.uint32
AX = mybir.AxisListType
OP = mybir.AluOpType
AF = mybir.ActivationFunctionType

# which column-chunks run their pack-STT on the vector engine (rest on gpsimd)
STT_ON_VECTOR = tuple(range(16))


def build_nc(debug_out: bool = False):
    nc = bacc.Bacc("TRN2", target_bir_lowering=False, debug=False)

    rhsT_d = nc.dram_tensor("rhsT", [514, N], BF16, kind="ExternalInput")
    lhsT_d = nc.dram_tensor("lhsT", [514, SHARD], BF16, kind="ExternalInput")
    egat_d = nc.dram_tensor("egather", [N, D], BF16, kind="ExternalInput")
    iota_d = nc.dram_tensor("iota", [128, 512], F16, kind="ExternalInput")
    bias_d = nc.dram_tensor("bias", [128, RT], F32, kind="ExternalInput")
    self_d = nc.dram_tensor("selfidx", [128, RT], F32, kind="ExternalInput")
    refc_d = nc.dram_tensor("refc", [SHARD, 16], F32, kind="ExternalInput")
    refa_d = nc.dram_tensor("refa", [SHARD, 112], F32, kind="ExternalInput")
    triu_d = nc.dram_tensor("triu", [128, 7], I16, kind="ExternalInput")
    part_d = nc.dram_tensor("partial", [1, 2], F32, kind="ExternalOutput")
    if debug_out:
        dbg_idx_d = nc.dram_tensor("dbg_idx", [128, 16], F32, kind="ExternalOutput")
        dbg_d2_d = nc.dram_tensor("dbg_d2", [128, 16], F32, kind="ExternalOutput")
        dbg_srtc_d = nc.dram_tensor("dbg_srtc", [128, 16], F32, kind="ExternalOutput")
        dbg_ang_d = nc.dram_tensor("dbg_ang", [128, 112], F32, kind="ExternalOutput")
        dbg_cand_d = nc.dram_tensor("dbg_cand", [128, 128], F32, kind="ExternalOutput")

    # preamble (before Tile body): the gather-count register, so its
    # RegisterMove precedes every Tile-scheduled consumer on gpsimd
    r2048 = nc.gpsimd.to_reg(2048)
    rfill1 = nc.gpsimd.to_reg(1.0)

    with tile.TileContext(nc) as tc, ExitStack() as ctx:
        const = ctx.enter_context(tc.tile_pool(name="const", bufs=1))
        sel = ctx.enter_context(tc.tile_pool(name="sel", bufs=3))
        scr = ctx.enter_context(tc.tile_pool(name="scr", bufs=4))
        gbuf = ctx.enter_context(tc.tile_pool(name="gbuf", bufs=1))
        vbuf = ctx.enter_context(tc.tile_pool(name="vbuf", bufs=2))
        gbuf2 = ctx.enter_context(tc.tile_pool(name="gbuf2", bufs=2))
        psS = ctx.enter_context(tc.tile_pool(name="psS", bufs=2, space="PSUM"))
        psG = ctx.enter_context(tc.tile_pool(name="psG", bufs=2, space="PSUM"))

        # ---- constants / resident data ----
        rhs_sb = [const.tile([128, N], BF16, tag=f"rhs{c}", name=f"rhs{c}") for c in range(4)]
        rhs_aug = const.tile([2, N], BF16, tag="rhsaug")
        lhs_sb = [const.tile([128, SHARD], BF16, tag=f"lhs{c}", name=f"lhs{c}") for c in range(4)]
        lhs_aug = const.tile([2, SHARD], BF16, tag="lhsaug")
        iota_sb = const.tile([128, 512], F16, tag="iota")
        bias_sb = const.tile([128, RT], F32, tag="bias")
        self_sb = const.tile([128, RT], F32, tag="self")
        triu_sb = const.tile([128, 7], I16, tag="triu")
        ident = const.tile([128, 128], F32, tag="ident")
        perm = const.tile([128, 128], F32, tag="perm")
        repmat = const.tile([16, 128], F32, tag="repmat")
        css = const.tile([128, 1], F32, tag="css")
        ass = const.tile([128, 1], F32, tag="ass")

        nc.sync.dma_start(iota_sb[:], iota_d.ap()[:])
        nc.sync.dma_start(bias_sb[:], bias_d.ap()[:])
        nc.sync.dma_start(self_sb[:], self_d.ap()[:])
        nc.sync.dma_start(triu_sb[:], triu_d.ap()[:])
        nc.sync.dma_start(rhs_aug[:], rhsT_d.ap()[512:514, :])
        nc.sync.dma_start(lhs_aug[:], lhsT_d.ap()[512:514, :])
        for c in range(4):
            nc.sync.dma_start(lhs_sb[c][:], lhsT_d.ap()[ts(c, 128), :])
        # rhs in column chunks, first columns of every K-tile first, so the
        # first matmuls start as soon as ~1/8th of the data has landed
        for jj in range(8):
            for c in range(4):
                nc.sync.dma_start(
                    rhs_sb[c][:, ts(jj, 1024)],
                    rhsT_d.ap()[ts(c, 128), ts(jj, 1024)],
                )
        nc.gpsimd.memset(ident[:], 0.0)
        nc.gpsimd.affine_select(
            out=ident[:], in_=ident[:], compare_op=OP.not_equal,
            fill=rfill1, base=0, pattern=[[-1, 128]], channel_multiplier=1,
        )
        # perm[k, 8u+v] = 1 iff k == 16v+u  (gram-position permutation)
        nc.gpsimd.memset(perm[:], 0.0)
        nc.gpsimd.affine_select(
            out=bass.AP(tensor=perm[:].tensor, offset=0,
                        ap=[perm[:].ap[0], [8, 16], [1, 8]]),
            in_=bass.AP(tensor=perm[:].tensor, offset=0,
                        ap=[perm[:].ap[0], [8, 16], [1, 8]]),
            compare_op=OP.not_equal,
            fill=rfill1, base=0,
            pattern=[[-1, 16], [-16, 8]],
            channel_multiplier=1,
        )
        # repmat[k, 16a+s] = 1 iff k == s  (partition-block replicator)
        nc.gpsimd.memset(repmat[:], 0.0)
        nc.gpsimd.affine_select(
            out=bass.AP(tensor=repmat[:].tensor, offset=0,
                        ap=[repmat[:].ap[0], [16, 8], [1, 16]]),
            in_=bass.AP(tensor=repmat[:].tensor, offset=0,
                        ap=[repmat[:].ap[0], [16, 8], [1, 16]]),
            compare_op=OP.not_equal,
            fill=rfill1, base=0,
            pattern=[[0, 8], [-1, 16]],
            channel_multiplier=1,
        )
        nc.vector.memset(css[:], 0.0)
        nc.vector.memset(ass[:], 0.0)

        def lhs_slice(k, t):
            if k < 4:
                return lhs_sb[k][:, ts(t, 128)]
            return lhs_aug[:, ts(t, 128)]

        def rhs_slice(k, j):
            if k < 4:
                return rhs_sb[k][:, ts(j, 512)]
            return rhs_aug[:, ts(j, 512)]

        pending = []
        prev_gate = [None]
        gcopies = [None]
        HALVES = [(0, 5), (5, 3)]
        for half, (tbase, tbh) in enumerate(HALVES):
            gstack = gbuf2.tile([128, 16 * tbh * 16], F16, tag="gstack")
            invd_st = gbuf2.tile([128, 8, 16], F32, tag="invdst")
            for tt in range(tbh):
                t = tbase + tt
                # ================= phase A: matmul + pack + chunk-max8 ====
                cand = sel.tile([128, 128], F32, tag="cand")
                for jg in range(NCH // 2):
                    pss = [
                        psS.tile([128, 512], F32, tag="psA", name="psA"),
                        psS.tile([128, 512], F32, tag="psB", name="psB"),
                    ]
                    for k in range(5):
                        for q in range(2):
                            nc.tensor.matmul(
                                pss[q][:],
                                lhs_slice(k, t),
                                rhs_slice(k, 2 * jg + q),
                                start=(k == 0),
                                stop=(k == 4),
                            )
                    for q in range(2):
                        j = 2 * jg + q
                        t1 = scr.tile([128, 512], F32, tag="t1")
                        relu_i = nc.scalar.activation(
                            t1[:], pss[q][:], AF.Relu,
                            bias=bias_sb[:, t : t + 1], scale=QSCALE,
                        )
                        if jg == 0 and q == 1 and gcopies[0]:
                            rdep = InstructionNameOrderedSet()
                            rdep.add(relu_i.ins.name)
                            for cp in gcopies[0]:
                                cp.ins.add_nosync_dependencies_from(rdep)
                            gcopies[0] = None
                        t2 = scr.tile([128, 512], F32, tag="t2")
                        eng = nc.vector if j in STT_ON_VECTOR else nc.gpsimd
                        stt_i = eng.scalar_tensor_tensor(
                            t2[:], t1[:], -MAGIC, iota_sb[:],
                            op0=OP.add, op1=OP.add,
                        )
                        if prev_gate[0] is not None:
                            stt_i.ins.add_nosync_dependencies_from(prev_gate[0])
                        nc.vector.max(cand[:, ts(j, 8)], t2[:])

                if pending:
                    pending.pop(0)()
                # ================= phase B: select top-16 + unpack ========
                v16 = sel.tile([128, 16], F32, tag="v16")
                nc.vector.max(v16[:, 0:8], cand[:])
                candz = sel.tile([128, 128], F32, tag="candz")
                nc.vector.match_replace(candz[:], v16[:, 0:8], cand[:], NEG_BIG)
                nc.vector.max(v16[:, 8:16], candz[:])
                pos = sel.tile([128, 16], U32, tag="pos")
                nc.vector.max_index(pos[:, 0:8], v16[:, 0:8], cand[:])
                nc.vector.max_index(pos[:, 8:16], v16[:, 8:16], candz[:])
                chunk_u = sel.tile([128, 16], U32, tag="chunku")
                nc.vector.tensor_scalar(
                    chunk_u[:], pos[:], 3, None, op0=OP.logical_shift_right
                )
                chunk_f = sel.tile([128, 16], F32, tag="chunkf")
                nc.vector.tensor_copy(chunk_f[:], chunk_u[:])
                # unpack m (integer part) via magic round
                s1 = sel.tile([128, 16], F32, tag="s1")
                nc.vector.tensor_scalar(
                    s1[:], v16[:], -0.4990234375, None, op0=OP.add
                )
                wv = sel.tile([128, 16], F32, tag="wv")
                nc.scalar.activation(wv[:], s1[:], AF.Copy, bias=MAGIC, scale=1.0)
                m16 = sel.tile([128, 16], F32, tag="m16")
                nc.vector.tensor_scalar(m16[:], wv[:], -MAGIC, None, op0=OP.add)
                # frac = v16 - m16 = idx/512 ; gidx = (chunk + frac) * 512
                frac = sel.tile([128, 16], F32, tag="frac")
                nc.vector.scalar_tensor_tensor(
                    frac[:], m16[:], -1.0, v16[:], op0=OP.mult, op1=OP.add
                )
                gidx = sel.tile([128, 16], F32, tag="gidx")
                nc.vector.tensor_tensor(gidx[:], chunk_f[:], frac[:], op=OP.add)
                nc.vector.tensor_scalar(gidx[:], gidx[:], 512.0, None, op0=OP.mult)
                # dp2 = C0 - m/QSCALE (slot 0 = self, dropped)
                dp2 = sel.tile([128, 16], F32, tag="dp2")
                nc.vector.tensor_scalar(
                    dp2[:], m16[:], -1.0 / QSCALE, C0, op0=OP.mult, op1=OP.add
                )
                # ================= phase C: curvature ======================
                d2re = sel.tile([128, 16], F32, tag="d2re")
                nc.vector.tensor_scalar_max(d2re[:, 0:15], dp2[:, 1:16], 1e-12)
                nc.vector.memset(d2re[:, 15:16], 1.0)
                dt_ = sel.tile([128, 16], F32, tag="dt")
                nc.scalar.sqrt(dt_[:], d2re[:])
                nc.vector.reciprocal(invd_st[:, tt, :], dt_[:])
                dsum = sel.tile([128, 1], F32, tag="dsum")
                nc.vector.reduce_sum(dsum[:], dt_[:, 0:15], axis=AX.X)
                dmean = sel.tile([128, 1], F32, tag="dmean")
                nc.vector.tensor_scalar(
                    dmean[:], dsum[:], 1.0 / 15.0, 1e-8, op0=OP.mult, op1=OP.add
                )
                ivm = sel.tile([128, 1], F32, tag="ivm")
                nc.vector.reciprocal(ivm[:], dmean[:])
                sig = sel.tile([128, 16], F32, tag="sig")
                nc.vector.tensor_scalar(
                    sig[:, 0:15], dt_[:, 0:15], ivm[:], None, op0=OP.mult
                )
                nc.vector.memset(sig[:, 15:16], PAD_CURV)
                srtc = sel.tile([128, 16], F32, tag="srtc")
                nc.vector.max(srtc[:, 0:8], sig[:])
                sigz = sel.tile([128, 16], F32, tag="sigz")
                nc.vector.match_replace(sigz[:], srtc[:, 0:8], sig[:], -2.0)
                nc.vector.max(srtc[:, 8:16], sigz[:])
                refc_t = scr.tile([128, 16], F32, tag="refct")
                nc.scalar.dma_start(refc_t[:], refc_d.ap()[ts(t, 128), :])
                dcv = sel.tile([128, 16], F32, tag="dcv")
                nc.vector.tensor_tensor(
                    dcv[:], srtc[:], refc_t[:], op=OP.subtract
                )
                csq = sel.tile([128, 16], F32, tag="csq")
                css_t = sel.tile([128, 1], F32, tag="csst")
                nc.scalar.activation(csq[:], dcv[:], AF.Square, accum_out=css_t[:])
                nc.vector.tensor_tensor(css[:], css[:], css_t[:], op=OP.add)
                # ================= phase D: gather + gram ==================
                kif = sel.tile([128, 16], F32, tag="kif")
                nc.vector.tensor_copy(kif[:, 0:15], gidx[:, 1:16])
                nc.vector.tensor_copy(kif[:, 15:16], self_sb[:, t : t + 1])
                # idx16[16a+s, Q] = kif[phi(Q), s]: transpose (perm rhs),
                # then replicate the 16-row block via a constant matmul.
                pst1 = psS.tile([16, 128], F32, tag="psB", name="pst1")
                nc.tensor.transpose(pst1[:], kif[:], perm[:])
                t1s = sel.tile([16, 128], F32, tag="t1s")
                nc.vector.tensor_copy(t1s[:], pst1[:])
                pst = psS.tile([128, 128], F32, tag="pstT", name="pst")
                nc.tensor.matmul(pst[:], repmat[:], t1s[:], start=True, stop=True)
                idx16 = sel.tile([128, 128], I16, tag="idx16")
                idx16_i = nc.vector.tensor_copy(idx16[:], pst[:])
                gate = InstructionNameOrderedSet()
                gate.add(idx16_i.ins.name)
                prev_gate[0] = gate
                vt = vbuf.tile([128, 4, 2048], BF16, tag="vt")
                nc.gpsimd.dma_gather(
                    out_ap=vt[:],
                    in_ap=egat_d.ap()[:],
                    idxs_ap=idx16[:],
                    num_idxs=2048,
                    num_idxs_reg=r2048,
                    elem_size=512,
                    transpose=True,
                    single_packet=False,
                )

                def gram_phase(tt=tt, vt=vt, gstack=gstack, tbh=tbh):
                    gcop = []
                    gcopies[0] = gcop
                    gfs = 16 * tbh * 16
                    gsk_t = scr.tile([128, 2048], F16, tag="gskt", name="gskt")
                    for w2 in range(4):
                        pg = psG.tile([128, 512], F32, tag="pg", name="pg")
                        for g2h in range(4):
                            g2 = 4 * w2 + g2h
                            for c in range(4):
                                nc.tensor.matmul(
                                    pg[:, ts(g2h, 128)],
                                    vt[:, c, ts(g2, 128)],
                                    vt[:, c, ts(g2, 128)],
                                    start=(c == 0),
                                    stop=(c == 3),
                                )
                        cp_i = nc.scalar.activation(
                            gsk_t[:, ts(w2, 512)], pg[:], AF.Copy)
                        gcop.append(cp_i)
                    # hop-1 batched over all 16 gram groups (one DMA per block p):
                    # gstack[16p+l, (g2*tbh+tt)*16+m] = gsk[16p+l, 128*g2+16p+m]
                    for p in range(8):
                        h1_src = bass.AP(
                            tensor=gsk_t[:].tensor,
                            offset=gsk_t[:].offset + 16 * p * 2048 + 16 * p,
                            ap=[[2048, 16], [128, 16], [1, 16]],
                        )
                        h1_dst = bass.AP(
                            tensor=gstack[:].tensor,
                            offset=gstack[:].offset + 16 * p * gfs + tt * 16,
                            ap=[[gfs, 16], [16 * tbh, 16], [1, 16]],
                        )
                        nc.sync.dma_start(h1_dst, h1_src)
                pending.append(gram_phase)
                if debug_out and t == 0:
                    nc.sync.dma_start(dbg_idx_d.ap()[:], kif[:])
                    nc.sync.dma_start(dbg_d2_d.ap()[:], dp2[:])
                    nc.sync.dma_start(dbg_srtc_d.ap()[:], srtc[:])
                    nc.sync.dma_start(dbg_cand_d.ap()[:], cand[:])

            while pending:
                pending.pop(0)()
            # ================= phase E: fold DMAs ======================
            ptR = gbuf2.tile([128, 16 * tbh * 16], F16, tag="ptR")
            FSg = 16 * tbh * 16
            FSp = FSg
            TBM = tbh * 16
            for p in range(8):
                for l in range(16):
                    src = bass.AP(
                        tensor=gstack[:].tensor,
                        offset=gstack[:].offset + (16 * p + l) * FSg,
                        ap=[[FSg, 1], [TBM, 16], [1, TBM]],
                    )
                    dst = bass.AP(
                        tensor=ptR[:].tensor,
                        offset=ptR[:].offset + 16 * p * FSp + l * TBM,
                        ap=[[FSp, 16], [1, TBM]],
                    )
                    eng = nc.sync if (p * 16 + l) % 2 == 0 else nc.scalar
                    eng.dma_start(dst, src)

            # ================= phase F: cos + sort + MSE ===============
            for tt in range(tbh):
                t = tbase + tt
                prt = ptR[:]
                p0 = prt.ap[0]
                base = prt.offset + tt * 16
                pr = bass.AP(tensor=prt.tensor, offset=base,
                             ap=[p0, [TBM, 16], [1, 16]])
                # raw = R - R[l,15] - R[15,m] + R[15,15]
                in_l15 = bass.AP(tensor=prt.tensor, offset=base + 15,
                                 ap=[p0, [TBM, 16], [0, 16]])
                in_r15 = bass.AP(tensor=prt.tensor, offset=base + 15 * TBM,
                                 ap=[p0, [0, 16], [1, 16]])
                in_r15b = bass.AP(tensor=prt.tensor, offset=base + 15 * TBM,
                                  ap=[p0, [1, 16]])
                in_r1515 = bass.AP(tensor=prt.tensor,
                                   offset=base + 15 * TBM + 15,
                                   ap=[p0, [0, 16]])
                ta = sel.tile([128, 256], F32, tag="ta")
                nc.vector.tensor_tensor(ta[:], pr, in_l15, op=OP.subtract)
                tb2 = sel.tile([128, 16], F32, tag="tb2")
                nc.vector.tensor_tensor(tb2[:], in_r15b, in_r1515, op=OP.subtract)
                tb2b = bass.AP(tensor=tb2[:].tensor, offset=tb2[:].offset,
                               ap=[tb2[:].ap[0], [0, 16], [1, 16]])
                tbb = sel.tile([128, 256], F32, tag="tbb")
                nc.vector.tensor_tensor(tbb[:], ta[:], tb2b, op=OP.subtract)
                ivt = invd_st[:, tt, :]
                iv_l = bass.AP(
                    tensor=ivt.tensor, offset=ivt.offset,
                    ap=[ivt.ap[0], [1, 16], [0, 16]],
                )
                iv_m = bass.AP(
                    tensor=ivt.tensor, offset=ivt.offset,
                    ap=[ivt.ap[0], [0, 16], [1, 16]],
                )
                tcc = sel.tile([128, 256], F32, tag="tc")
                nc.vector.tensor_tensor(tcc[:], tbb[:], iv_l, op=OP.mult)
                cosv = sel.tile([128, 256], F32, tag="cosv")
                nc.vector.tensor_tensor(cosv[:], tcc[:], iv_m, op=OP.mult)
                angv = sel.tile([128, 112], F32, tag="angv")
                if os.environ.get('K_NO_APGATHER') == '1':
                    nc.vector.memset(angv[:], 0.5)
                elif True:
                    nc.gpsimd.ap_gather(
                    out_ap=angv[:].rearrange("p (a b) -> p a b", b=1),
                    in_ap=cosv[:].rearrange("p (a b) -> p a b", b=1),
                    idxs_ap=triu_sb[:],
                    channels=128,
                    num_elems=256,
                    d=1,
                    num_idxs=112,
                )
                nc.vector.memset(angv[:, 105:112], PAD_ANG)
                srta = sel.tile([128, 112], F32, tag="srta")
                work = angv
                for r in range(14):
                    nc.vector.max(srta[:, ts(r, 8)], work[:])
                    if r < 13:
                        nwork = sel.tile([128, 112], F32, tag=f"work{r % 2}")
                        nc.vector.match_replace(
                            nwork[:], srta[:, ts(r, 8)], work[:], NEG_BIG
                        )
                        work = nwork
                refa_t = scr.tile([128, 112], F32, tag="refat")
                nc.scalar.dma_start(refa_t[:], refa_d.ap()[ts(t, 128), :])
                dav = sel.tile([128, 112], F32, tag="dav")
                nc.vector.tensor_tensor(dav[:], srta[:], refa_t[:], op=OP.subtract)
                asq = sel.tile([128, 112], F32, tag="asq")
                ass_t = sel.tile([128, 1], F32, tag="asst")
                nc.scalar.activation(asq[:], dav[:], AF.Square, accum_out=ass_t[:])
                nc.vector.tensor_tensor(ass[:], ass[:], ass_t[:], op=OP.add)
                if debug_out and t == 0:
                    nc.sync.dma_start(dbg_ang_d.ap()[:], srta[:])

        # ---- final reduce + output ----
        cred = const.tile([128, 1], F32, tag="cred")
        ared = const.tile([128, 1], F32, tag="ared")
        if os.environ.get('K_NO_PALLRED') == '1':
            nc.vector.tensor_copy(cred[:], css[:])
            nc.vector.tensor_copy(ared[:], ass[:])
        else:
            nc.gpsimd.partition_all_reduce(
                cred[:], css[:], channels=128, reduce_op=bass_isa.ReduceOp.add
            )
            nc.gpsimd.partition_all_reduce(
                ared[:], ass[:], channels=128, reduce_op=bass_isa.ReduceOp.add
            )
        fin = const.tile([1, 2], F32, tag="fin")
        nc.vector.tensor_copy(fin[0:1, 0:1], cred[0:1, :])
        nc.vector.tensor_copy(fin[0:1, 1:2], ared[0:1, :])
        nc.sync.dma_start(part_d.ap()[:], fin[:])

    nc.compile()
    return nc


# =====================  host side  =====================

def _prep_inputs(embeddings, reference_curvature_sig, reference_angular_sig):
    emb32 = np.asarray(embeddings, dtype=np.float32)
    refc = np.asarray(reference_curvature_sig, dtype=np.float32)
    refa = np.asarray(reference_angular_sig, dtype=np.float32)

    e1_bf = (np.float32(np.sqrt(2.0)) * emb32).astype(ml_dtypes.bfloat16)
    e1 = e1_bf.astype(np.float32)
    e2_bf = (2.0 * e1).astype(ml_dtypes.bfloat16)       # exact x2
    n1 = np.sum(e1.astype(np.float64) * e1.astype(np.float64), axis=1).astype(
        np.float32
    )
    nnhi = n1.astype(ml_dtypes.bfloat16).astype(np.float32)
    nnlo = (n1 - nnhi).astype(ml_dtypes.bfloat16).astype(np.float32)

    rhsT = np.empty((514, N), dtype=ml_dtypes.bfloat16)
    rhsT[:512] = e2_bf.T
    rhsT[512] = (-nnhi).astype(ml_dtypes.bfloat16)
    rhsT[513] = (-nnlo).astype(ml_dtypes.bfloat16)

    lhsT_full = np.empty((514, N), dtype=ml_dtypes.bfloat16)
    lhsT_full[:512] = e1_bf.T
    lhsT_full[512:] = np.float32(1.0)

    iota = ((np.arange(512, dtype=np.float32) / 512.0)[None, :]
            .repeat(128, axis=0).astype(np.float16))

    tri = [l * 16 + m for l in range(15) for m in range(l + 1, 15)]
    tri += [255] * 7
    tri = np.array(tri, dtype=np.int16)                 # 112 entries
    triu = np.zeros((128, 7), dtype=np.int16)
    for p in range(128):
        for s in range(7):
            triu[p, s] = tri[s * 16 + (p & 15)]

    shared = dict(
        rhsT=rhsT, egather=e1_bf, iota=iota, triu=triu,
    )
    per_core = []
    for c in range(NCORES):
        lo = c * SHARD
        sl = slice(lo, lo + SHARD)
        bias = (MAGIC + (C0 - n1[sl].astype(np.float64)) * QSCALE).astype(np.float32)
        bias_t = bias.reshape(RT, 128).T.copy()         # [128, RT]
        selfidx = (np.arange(lo, lo + SHARD, dtype=np.float32)
                   .reshape(RT, 128).T.copy())
        refc_c = np.full((SHARD, 16), PAD_CURV, dtype=np.float32)
        refc_c[:, 0:15] = refc[sl, ::-1]
        refa_c = np.full((SHARD, 112), PAD_ANG, dtype=np.float32)
        refa_c[:, 0:105] = refa[sl, ::-1]
        per_core.append(dict(
            shared,
            lhsT=np.ascontiguousarray(lhsT_full[:, sl]),
            bias=bias_t,
            selfidx=selfidx,
            refc=refc_c,
            refa=refa_c,
        ))
    return per_core


_NC_CACHE = {}


def run_cores(inputs, debug_out=False, **run_kwargs):
    key = debug_out
    if key not in _NC_CACHE:
        _NC_CACHE[key] = build_nc(debug_out=debug_out)
    nc = _NC_CACHE[key]
    in_maps = _prep_inputs(**inputs)
    res = run_bass_kernel_spmd(
        nc, in_maps, core_ids=list(range(NCORES)), **run_kwargs
    )
    return res


def kernel(embeddings, reference_curvature_sig, reference_angular_sig):
    res = run_cores(dict(
        embeddings=embeddings,
        reference_curvature_sig=reference_curvature_sig,
        reference_angular_sig=reference_angular_sig,
    ))
    css = 0.0
    ass = 0.0
    for r in res.results:
        css += float(r["partial"][0, 0])
        ass += float(r["partial"][0, 1])
    curv_loss = css / (N * 15)
    ang_loss = ass / (N * 105)
    out = np.float32(0.3 * curv_loss + 0.7 * ang_loss)
    return np.asarray(out, dtype=np.float32)



# revision 42
# speedup vs baseline: 1.2760x; 1.2760x over previous
"""Trainium2 Bass kernel for nn_CurvatureOnlyRegularizer (retrieval_knn).

Full inputs -> full output. Shards the 8192 points row-wise across 8 cores.

Per-core pipeline (1024 rows = 8 row-tiles of 128):
  1. S* = e1 . e2^T - n_j via bf16 PE matmul with 2 augmented K-rows carrying
     the -n_j (split hi/lo bf16) term. e1 = bf16(sqrt2*emb), e2 = 2*e1.
  2. ACT evacuates PSUM as t1 = Relu(psum*25 + bias_i) where bias_i folds a
     per-row offset and the 1.5*2^23 magic constant, quantizing -d'^2 to a
     14-bit integer m in fp32's integer binade.
  3. One fused scalar_tensor_tensor pass packs t2 = m + idx_local/512 (iota is
     a preloaded fp16 tile, exact). Per-512-chunk max8 then carries indices
     through selection for free; top-16-of-candidates + max_index on the
     128-wide candidate array recovers (chunk, local) -> global idx.
  4. Neighbor embeddings gathered via dma_gather(transpose=True) directly into
     [D-partition, point*16] layout; PE gram (4 K-chunks) gives all pairwise
     dots incl. the self row/col, so v_l.v_m = G[l,m]-G[l,15]-G[15,m]+G[15,15]
     without ever materializing v.
  5. A 128-DMA "fold" converts block-diagonal G (fp16) to point-major rows.
  6. cos = raw * invd_l * invd_m with d' from the quantized m; upper-triangle
     extracted by ap_gather; signatures sorted descending by max8/match_replace
     rounds and MSE'd against host-reversed reference signatures on ACT.
Host sums the 8 per-core partial sums.
"""

import os
from contextlib import ExitStack

import ml_dtypes
import numpy as np

import concourse.bass as bass
import concourse.bass_isa as bass_isa
import concourse.mybir as mybir
import concourse.tile as tile
from concourse import bacc
from concourse.bass import ds, ts
from concourse.bass_utils import run_bass_kernel_spmd
from concourse.instruction_name_ordered_set import InstructionNameOrderedSet
from concourse.masks import make_identity

N, D, K = 8192, 512, 15
NCORES = 8
SHARD = N // NCORES            # 1024
RT = SHARD // 128              # 8 row-tiles per core
NCH = N // 512                 # 16 column chunks
MAGIC = 12582912.0             # 1.5 * 2^23
C0 = 2052.0
QSCALE = 25.0
PAD_CURV = -1.0
PAD_ANG = -4.0
NEG_BIG = -3.0e38
F32 = mybir.dt.float32
F16 = mybir.dt.float16
BF16 = mybir.dt.bfloat16
I16 = mybir.dt.int16
U32 = mybir.dt.uint32
AX = mybir.AxisListType
OP = mybir.AluOpType
AF = mybir.ActivationFunctionType

# which column-chunks run their pack-STT on the vector engine (rest on gpsimd)
STT_ON_VECTOR = tuple(range(16))


def build_nc(debug_out: bool = False):
    nc = bacc.Bacc("TRN2", target_bir_lowering=False, debug=False)

    rhsT_d = nc.dram_tensor("rhsT", [514, N], BF16, kind="ExternalInput")
    lhsT_d = nc.dram_tensor("lhsT", [514, SHARD], BF16, kind="ExternalInput")
    egat_d = nc.dram_tensor("egather", [N, D], BF16, kind="ExternalInput")
    iota_d = nc.dram_tensor("iota", [128, 512], F16, kind="ExternalInput")
    bias_d = nc.dram_tensor("bias", [128, RT], F32, kind="ExternalInput")
    self_d = nc.dram_tensor("selfidx", [128, RT], F32, kind="ExternalInput")
    refc_d = nc.dram_tensor("refc", [SHARD, 16], F32, kind="ExternalInput")
    refa_d = nc.dram_tensor("refa", [SHARD, 112], F32, kind="ExternalInput")
    triu_d = nc.dram_tensor("triu", [128, 7], I16, kind="ExternalInput")
    part_d = nc.dram_tensor("partial", [1, 2], F32, kind="ExternalOutput")
    if debug_out:
        dbg_idx_d = nc.dram_tensor("dbg_idx", [128, 16], F32, kind="ExternalOutput")
        dbg_d2_d = nc.dram_tensor("dbg_d2", [128, 16], F32, kind="ExternalOutput")
        dbg_srtc_d = nc.dram_tensor("dbg_srtc", [128, 16], F32, kind="ExternalOutput")
        dbg_ang_d = nc.dram_tensor("dbg_ang", [128, 112], F32, kind="ExternalOutput")
        dbg_cand_d = nc.dram_tensor("dbg_cand", [128, 128], F32, kind="ExternalOutput")

    # preamble (before Tile body): the gather-count register, so its
    # RegisterMove precedes every Tile-scheduled consumer on gpsimd
    r2048 = nc.gpsimd.to_reg(2048)
    rfill1 = nc.gpsimd.to_reg(1.0)

    with tile.TileContext(nc) as tc, ExitStack() as ctx:
        const = ctx.enter_context(tc.tile_pool(name="const", bufs=1))
        sel = ctx.enter_context(tc.tile_pool(name="sel", bufs=3))
        scr = ctx.enter_context(tc.tile_pool(name="scr", bufs=4))
        gbuf = ctx.enter_context(tc.tile_pool(name="gbuf", bufs=1))
        vbuf = ctx.enter_context(tc.tile_pool(name="vbuf", bufs=2))
        gbuf2 = ctx.enter_context(tc.tile_pool(name="gbuf2", bufs=2))
        psS = ctx.enter_context(tc.tile_pool(name="psS", bufs=2, space="PSUM"))
        psG = ctx.enter_context(tc.tile_pool(name="psG", bufs=2, space="PSUM"))

        # ---- constants / resident data ----
        rhs_sb = [const.tile([128, N], BF16, tag=f"rhs{c}", name=f"rhs{c}") for c in range(4)]
        rhs_aug = const.tile([2, N], BF16, tag="rhsaug")
        lhs_sb = [const.tile([128, SHARD], BF16, tag=f"lhs{c}", name=f"lhs{c}") for c in range(4)]
        lhs_aug = const.tile([2, SHARD], BF16, tag="lhsaug")
        iota_sb = const.tile([128, 512], F16, tag="iota")
        bias_sb = const.tile([128, RT], F32, tag="bias")
        self_sb = const.tile([128, RT], F32, tag="self")
        triu_sb = const.tile([128, 7], I16, tag="triu")
        ident = const.tile([128, 128], F32, tag="ident")
        perm = const.tile([128, 128], F32, tag="perm")
        repmat = const.tile([16, 128], F32, tag="repmat")
        css = const.tile([128, 1], F32, tag="css")
        ass = const.tile([128, 1], F32, tag="ass")

        nc.sync.dma_start(iota_sb[:], iota_d.ap()[:])
        nc.sync.dma_start(bias_sb[:], bias_d.ap()[:])
        nc.sync.dma_start(self_sb[:], self_d.ap()[:])
        nc.sync.dma_start(triu_sb[:], triu_d.ap()[:])
        nc.sync.dma_start(rhs_aug[:], rhsT_d.ap()[512:514, :])
        nc.sync.dma_start(lhs_aug[:], lhsT_d.ap()[512:514, :])
        for c in range(4):
            nc.sync.dma_start(lhs_sb[c][:], lhsT_d.ap()[ts(c, 128), :])
        # rhs in column chunks, first columns of every K-tile first, so the
        # first matmuls start as soon as ~1/8th of the data has landed
        for jj in range(8):
            for c in range(4):
                nc.sync.dma_start(
                    rhs_sb[c][:, ts(jj, 1024)],
                    rhsT_d.ap()[ts(c, 128), ts(jj, 1024)],
                )
        nc.gpsimd.memset(ident[:], 0.0)
        nc.gpsimd.affine_select(
            out=ident[:], in_=ident[:], compare_op=OP.not_equal,
            fill=rfill1, base=0, pattern=[[-1, 128]], channel_multiplier=1,
        )
        # perm[k, 8u+v] = 1 iff k == 16v+u  (gram-position permutation)
        nc.gpsimd.memset(perm[:], 0.0)
        nc.gpsimd.affine_select(
            out=bass.AP(tensor=perm[:].tensor, offset=0,
                        ap=[perm[:].ap[0], [8, 16], [1, 8]]),
            in_=bass.AP(tensor=perm[:].tensor, offset=0,
                        ap=[perm[:].ap[0], [8, 16], [1, 8]]),
            compare_op=OP.not_equal,
            fill=rfill1, base=0,
            pattern=[[-1, 16], [-16, 8]],
            channel_multiplier=1,
        )
        # repmat[k, 16a+s] = 1 iff k == s  (partition-block replicator)
        nc.gpsimd.memset(repmat[:], 0.0)
        nc.gpsimd.affine_select(
            out=bass.AP(tensor=repmat[:].tensor, offset=0,
                        ap=[repmat[:].ap[0], [16, 8], [1, 16]]),
            in_=bass.AP(tensor=repmat[:].tensor, offset=0,
                        ap=[repmat[:].ap[0], [16, 8], [1, 16]]),
            compare_op=OP.not_equal,
            fill=rfill1, base=0,
            pattern=[[0, 8], [-1, 16]],
            channel_multiplier=1,
        )
        nc.vector.memset(css[:], 0.0)
        nc.vector.memset(ass[:], 0.0)

        def lhs_slice(k, t):
            if k < 4:
                return lhs_sb[k][:, ts(t, 128)]
            return lhs_aug[:, ts(t, 128)]

        def rhs_slice(k, j):
            if k < 4:
                return rhs_sb[k][:, ts(j, 512)]
            return rhs_aug[:, ts(j, 512)]

        pending = []
        prev_gate = [None]
        gcopies = [None]
        gmms = [None]
        HALVES = [(0, 5), (5, 3)]
        for half, (tbase, tbh) in enumerate(HALVES):
            gstack = gbuf2.tile([128, 16 * tbh * 16], F16, tag="gstack")
            invd_st = gbuf2.tile([128, 8, 16], F32, tag="invdst")
            for tt in range(tbh):
                t = tbase + tt
                # ================= phase A: matmul + pack + chunk-max8 ====
                cand = sel.tile([128, 128], F32, tag="cand")
                for jg in range(NCH // 2):
                    pss = [
                        psS.tile([128, 512], F32, tag="psA", name="psA"),
                        psS.tile([128, 512], F32, tag="psB", name="psB"),
                    ]
                    for k in range(5):
                        for q in range(2):
                            mm_i = nc.tensor.matmul(
                                pss[q][:],
                                lhs_slice(k, t),
                                rhs_slice(k, 2 * jg + q),
                                start=(k == 0),
                                stop=(k == 4),
                            )
                    if jg == 1 and gmms[0]:
                        mdep = InstructionNameOrderedSet()
                        mdep.add(mm_i.ins.name)
                        for gm in gmms[0]:
                            gm.ins.add_nosync_dependencies_from(mdep)
                        gmms[0] = None
                    for q in range(2):
                        j = 2 * jg + q
                        t1 = scr.tile([128, 512], F32, tag="t1")
                        relu_i = nc.scalar.activation(
                            t1[:], pss[q][:], AF.Relu,
                            bias=bias_sb[:, t : t + 1], scale=QSCALE,
                        )
                        if jg == 0 and q == 1 and gcopies[0]:
                            rdep = InstructionNameOrderedSet()
                            rdep.add(relu_i.ins.name)
                            for cp in gcopies[0]:
                                cp.ins.add_nosync_dependencies_from(rdep)
                            gcopies[0] = None
                        t2 = scr.tile([128, 512], F32, tag="t2")
                        eng = nc.vector if j in STT_ON_VECTOR else nc.gpsimd
                        stt_i = eng.scalar_tensor_tensor(
                            t2[:], t1[:], -MAGIC, iota_sb[:],
                            op0=OP.add, op1=OP.add,
                        )
                        if prev_gate[0] is not None:
                            stt_i.ins.add_nosync_dependencies_from(prev_gate[0])
                        nc.vector.max(cand[:, ts(j, 8)], t2[:])

                if pending:
                    pending.pop(0)()
                # ================= phase B: select top-16 + unpack ========
                v16 = sel.tile([128, 16], F32, tag="v16")
                nc.vector.max(v16[:, 0:8], cand[:])
                candz = sel.tile([128, 128], F32, tag="candz")
                nc.vector.match_replace(candz[:], v16[:, 0:8], cand[:], NEG_BIG)
                nc.vector.max(v16[:, 8:16], candz[:])
                pos = sel.tile([128, 16], U32, tag="pos")
                nc.vector.max_index(pos[:, 0:8], v16[:, 0:8], cand[:])
                nc.vector.max_index(pos[:, 8:16], v16[:, 8:16], candz[:])
                chunk_u = sel.tile([128, 16], U32, tag="chunku")
                nc.vector.tensor_scalar(
                    chunk_u[:], pos[:], 3, None, op0=OP.logical_shift_right
                )
                chunk_f = sel.tile([128, 16], F32, tag="chunkf")
                nc.vector.tensor_copy(chunk_f[:], chunk_u[:])
                # unpack m (integer part) via magic round
                s1 = sel.tile([128, 16], F32, tag="s1")
                nc.vector.tensor_scalar(
                    s1[:], v16[:], -0.4990234375, None, op0=OP.add
                )
                wv = sel.tile([128, 16], F32, tag="wv")
                nc.scalar.activation(wv[:], s1[:], AF.Copy, bias=MAGIC, scale=1.0)
                m16 = sel.tile([128, 16], F32, tag="m16")
                nc.vector.tensor_scalar(m16[:], wv[:], -MAGIC, None, op0=OP.add)
                # frac = v16 - m16 = idx/512 ; gidx = (chunk + frac) * 512
                frac = sel.tile([128, 16], F32, tag="frac")
                nc.vector.scalar_tensor_tensor(
                    frac[:], m16[:], -1.0, v16[:], op0=OP.mult, op1=OP.add
                )
                gidx = sel.tile([128, 16], F32, tag="gidx")
                nc.vector.tensor_tensor(gidx[:], chunk_f[:], frac[:], op=OP.add)
                nc.vector.tensor_scalar(gidx[:], gidx[:], 512.0, None, op0=OP.mult)
                # dp2 = C0 - m/QSCALE (slot 0 = self, dropped)
                dp2 = sel.tile([128, 16], F32, tag="dp2")
                nc.vector.tensor_scalar(
                    dp2[:], m16[:], -1.0 / QSCALE, C0, op0=OP.mult, op1=OP.add
                )
                # ================= phase C: curvature ======================
                d2re = sel.tile([128, 16], F32, tag="d2re")
                nc.vector.tensor_scalar_max(d2re[:, 0:15], dp2[:, 1:16], 1e-12)
                nc.vector.memset(d2re[:, 15:16], 1.0)
                dt_ = sel.tile([128, 16], F32, tag="dt")
                nc.scalar.sqrt(dt_[:], d2re[:])
                nc.vector.reciprocal(invd_st[:, tt, :], dt_[:])
                dsum = sel.tile([128, 1], F32, tag="dsum")
                nc.vector.reduce_sum(dsum[:], dt_[:, 0:15], axis=AX.X)
                dmean = sel.tile([128, 1], F32, tag="dmean")
                nc.vector.tensor_scalar(
                    dmean[:], dsum[:], 1.0 / 15.0, 1e-8, op0=OP.mult, op1=OP.add
                )
                ivm = sel.tile([128, 1], F32, tag="ivm")
                nc.vector.reciprocal(ivm[:], dmean[:])
                sig = sel.tile([128, 16], F32, tag="sig")
                nc.vector.tensor_scalar(
                    sig[:, 0:15], dt_[:, 0:15], ivm[:], None, op0=OP.mult
                )
                nc.vector.memset(sig[:, 15:16], PAD_CURV)
                srtc = sel.tile([128, 16], F32, tag="srtc")
                nc.vector.max(srtc[:, 0:8], sig[:])
                sigz = sel.tile([128, 16], F32, tag="sigz")
                nc.vector.match_replace(sigz[:], srtc[:, 0:8], sig[:], -2.0)
                nc.vector.max(srtc[:, 8:16], sigz[:])
                refc_t = scr.tile([128, 16], F32, tag="refct")
                nc.scalar.dma_start(refc_t[:], refc_d.ap()[ts(t, 128), :])
                dcv = sel.tile([128, 16], F32, tag="dcv")
                nc.vector.tensor_tensor(
                    dcv[:], srtc[:], refc_t[:], op=OP.subtract
                )
                csq = sel.tile([128, 16], F32, tag="csq")
                css_t = sel.tile([128, 1], F32, tag="csst")
                nc.scalar.activation(csq[:], dcv[:], AF.Square, accum_out=css_t[:])
                nc.vector.tensor_tensor(css[:], css[:], css_t[:], op=OP.add)
                # ================= phase D: gather + gram ==================
                kif = sel.tile([128, 16], F32, tag="kif")
                nc.vector.tensor_copy(kif[:, 0:15], gidx[:, 1:16])
                nc.vector.tensor_copy(kif[:, 15:16], self_sb[:, t : t + 1])
                # idx16[16a+s, Q] = kif[phi(Q), s]: transpose (perm rhs),
                # then replicate the 16-row block via a constant matmul.
                pst1 = psS.tile([16, 128], F32, tag="psB", name="pst1")
                nc.tensor.transpose(pst1[:], kif[:], perm[:])
                t1s = sel.tile([16, 128], F32, tag="t1s")
                nc.vector.tensor_copy(t1s[:], pst1[:])
                pst = psS.tile([128, 128], F32, tag="pstT", name="pst")
                nc.tensor.matmul(pst[:], repmat[:], t1s[:], start=True, stop=True)
                idx16 = sel.tile([128, 128], I16, tag="idx16")
                idx16_i = nc.vector.tensor_copy(idx16[:], pst[:])
                gate = InstructionNameOrderedSet()
                gate.add(idx16_i.ins.name)
                prev_gate[0] = gate
                vt = vbuf.tile([128, 4, 2048], BF16, tag="vt")
                nc.gpsimd.dma_gather(
                    out_ap=vt[:],
                    in_ap=egat_d.ap()[:],
                    idxs_ap=idx16[:],
                    num_idxs=2048,
                    num_idxs_reg=r2048,
                    elem_size=512,
                    transpose=True,
                    single_packet=False,
                )

                def gram_phase(tt=tt, vt=vt, gstack=gstack, tbh=tbh):
                    gcop = []
                    gcopies[0] = gcop
                    gmm = []
                    gmms[0] = gmm
                    gfs = 16 * tbh * 16
                    gsk_t = scr.tile([128, 2048], F16, tag="gskt", name="gskt")
                    for w2 in range(4):
                        pg = psG.tile([128, 512], F32, tag="pg", name="pg")
                        for g2h in range(4):
                            g2 = 4 * w2 + g2h
                            for c in range(4):
                                gm_i = nc.tensor.matmul(
                                    pg[:, ts(g2h, 128)],
                                    vt[:, c, ts(g2, 128)],
                                    vt[:, c, ts(g2, 128)],
                                    start=(c == 0),
                                    stop=(c == 3),
                                )
                                gmm.append(gm_i)
                        cp_i = nc.scalar.activation(
                            gsk_t[:, ts(w2, 512)], pg[:], AF.Copy)
                        gcop.append(cp_i)
                    # hop-1 batched over all 16 gram groups (one DMA per block p):
                    # gstack[16p+l, (g2*tbh+tt)*16+m] = gsk[16p+l, 128*g2+16p+m]
                    for p in range(8):
                        h1_src = bass.AP(
                            tensor=gsk_t[:].tensor,
                            offset=gsk_t[:].offset + 16 * p * 2048 + 16 * p,
                            ap=[[2048, 16], [128, 16], [1, 16]],
                        )
                        h1_dst = bass.AP(
                            tensor=gstack[:].tensor,
                            offset=gstack[:].offset + 16 * p * gfs + tt * 16,
                            ap=[[gfs, 16], [16 * tbh, 16], [1, 16]],
                        )
                        nc.sync.dma_start(h1_dst, h1_src)
                pending.append(gram_phase)
                if debug_out and t == 0:
                    nc.sync.dma_start(dbg_idx_d.ap()[:], kif[:])
                    nc.sync.dma_start(dbg_d2_d.ap()[:], dp2[:])
                    nc.sync.dma_start(dbg_srtc_d.ap()[:], srtc[:])
                    nc.sync.dma_start(dbg_cand_d.ap()[:], cand[:])

            while pending:
                pending.pop(0)()
            # ================= phase E: fold DMAs ======================
            ptR = gbuf2.tile([128, 16 * tbh * 16], F16, tag="ptR")
            FSg = 16 * tbh * 16
            FSp = FSg
            TBM = tbh * 16
            for p in range(8):
                for l in range(16):
                    src = bass.AP(
                        tensor=gstack[:].tensor,
                        offset=gstack[:].offset + (16 * p + l) * FSg,
                        ap=[[FSg, 1], [TBM, 16], [1, TBM]],
                    )
                    dst = bass.AP(
                        tensor=ptR[:].tensor,
                        offset=ptR[:].offset + 16 * p * FSp + l * TBM,
                        ap=[[FSp, 16], [1, TBM]],
                    )
                    eng = nc.sync if half == 0 or (p * 16 + l) % 2 == 0 else nc.scalar
                    eng.dma_start(dst, src)

            # ================= phase F: cos + sort + MSE ===============
            for tt in range(tbh):
                t = tbase + tt
                prt = ptR[:]
                p0 = prt.ap[0]
                base = prt.offset + tt * 16
                pr = bass.AP(tensor=prt.tensor, offset=base,
                             ap=[p0, [TBM, 16], [1, 16]])
                # raw = R - R[l,15] - R[15,m] + R[15,15]
                in_l15 = bass.AP(tensor=prt.tensor, offset=base + 15,
                                 ap=[p0, [TBM, 16], [0, 16]])
                in_r15 = bass.AP(tensor=prt.tensor, offset=base + 15 * TBM,
                                 ap=[p0, [0, 16], [1, 16]])
                in_r15b = bass.AP(tensor=prt.tensor, offset=base + 15 * TBM,
                                  ap=[p0, [1, 16]])
                in_r1515 = bass.AP(tensor=prt.tensor,
                                   offset=base + 15 * TBM + 15,
                                   ap=[p0, [0, 16]])
                ta = sel.tile([128, 256], F32, tag="ta")
                nc.vector.tensor_tensor(ta[:], pr, in_l15, op=OP.subtract)
                tb2 = sel.tile([128, 16], F32, tag="tb2")
                nc.vector.tensor_tensor(tb2[:], in_r15b, in_r1515, op=OP.subtract)
                tb2b = bass.AP(tensor=tb2[:].tensor, offset=tb2[:].offset,
                               ap=[tb2[:].ap[0], [0, 16], [1, 16]])
                tbb = sel.tile([128, 256], F32, tag="tbb")
                nc.vector.tensor_tensor(tbb[:], ta[:], tb2b, op=OP.subtract)
                ivt = invd_st[:, tt, :]
                iv_l = bass.AP(
                    tensor=ivt.tensor, offset=ivt.offset,
                    ap=[ivt.ap[0], [1, 16], [0, 16]],
                )
                iv_m = bass.AP(
                    tensor=ivt.tensor, offset=ivt.offset,
                    ap=[ivt.ap[0], [0, 16], [1, 16]],
                )
                tcc = sel.tile([128, 256], F32, tag="tc")
                nc.vector.tensor_tensor(tcc[:], tbb[:], iv_l, op=OP.mult)
                cosv = sel.tile([128, 256], F32, tag="cosv")
                nc.vector.tensor_tensor(cosv[:], tcc[:], iv_m, op=OP.mult)
                angv = sel.tile([128, 112], F32, tag="angv")
                if os.environ.get('K_NO_APGATHER') == '1':
                    nc.vector.memset(angv[:], 0.5)
                elif True:
                    nc.gpsimd.ap_gather(
                    out_ap=angv[:].rearrange("p (a b) -> p a b", b=1),
                    in_ap=cosv[:].rearrange("p (a b) -> p a b", b=1),
                    idxs_ap=triu_sb[:],
                    channels=128,
                    num_elems=256,
                    d=1,
                    num_idxs=112,
                )
                nc.vector.memset(angv[:, 105:112], PAD_ANG)
                srta = sel.tile([128, 112], F32, tag="srta")
                work = angv
                for r in range(14):
                    nc.vector.max(srta[:, ts(r, 8)], work[:])
                    if r < 13:
                        nwork = sel.tile([128, 112], F32, tag=f"work{r % 2}")
                        nc.vector.match_replace(
                            nwork[:], srta[:, ts(r, 8)], work[:], NEG_BIG
                        )
                        work = nwork
                refa_t = scr.tile([128, 112], F32, tag="refat")
                nc.scalar.dma_start(refa_t[:], refa_d.ap()[ts(t, 128), :])
                dav = sel.tile([128, 112], F32, tag="dav")
                nc.vector.tensor_tensor(dav[:], srta[:], refa_t[:], op=OP.subtract)
                asq = sel.tile([128, 112], F32, tag="asq")
                ass_t = sel.tile([128, 1], F32, tag="asst")
                nc.scalar.activation(asq[:], dav[:], AF.Square, accum_out=ass_t[:])
                nc.vector.tensor_tensor(ass[:], ass[:], ass_t[:], op=OP.add)
                if debug_out and t == 0:
                    nc.sync.dma_start(dbg_ang_d.ap()[:], srta[:])

        # ---- final reduce + output ----
        cred = const.tile([128, 1], F32, tag="cred")
        ared = const.tile([128, 1], F32, tag="ared")
        if os.environ.get('K_NO_PALLRED') == '1':
            nc.vector.tensor_copy(cred[:], css[:])
            nc.vector.tensor_copy(ared[:], ass[:])
        else:
            nc.gpsimd.partition_all_reduce(
                cred[:], css[:], channels=128, reduce_op=bass_isa.ReduceOp.add
            )
            nc.gpsimd.partition_all_reduce(
                ared[:], ass[:], channels=128, reduce_op=bass_isa.ReduceOp.add
            )
        fin = const.tile([1, 2], F32, tag="fin")
        nc.vector.tensor_copy(fin[0:1, 0:1], cred[0:1, :])
        nc.vector.tensor_copy(fin[0:1, 1:2], ared[0:1, :])
        nc.sync.dma_start(part_d.ap()[:], fin[:])

    nc.compile()
    return nc


# =====================  host side  =====================

def _prep_inputs(embeddings, reference_curvature_sig, reference_angular_sig):
    emb32 = np.asarray(embeddings, dtype=np.float32)
    refc = np.asarray(reference_curvature_sig, dtype=np.float32)
    refa = np.asarray(reference_angular_sig, dtype=np.float32)

    e1_bf = (np.float32(np.sqrt(2.0)) * emb32).astype(ml_dtypes.bfloat16)
    e1 = e1_bf.astype(np.float32)
    e2_bf = (2.0 * e1).astype(ml_dtypes.bfloat16)       # exact x2
    n1 = np.sum(e1.astype(np.float64) * e1.astype(np.float64), axis=1).astype(
        np.float32
    )
    nnhi = n1.astype(ml_dtypes.bfloat16).astype(np.float32)
    nnlo = (n1 - nnhi).astype(ml_dtypes.bfloat16).astype(np.float32)

    rhsT = np.empty((514, N), dtype=ml_dtypes.bfloat16)
    rhsT[:512] = e2_bf.T
    rhsT[512] = (-nnhi).astype(ml_dtypes.bfloat16)
    rhsT[513] = (-nnlo).astype(ml_dtypes.bfloat16)

    lhsT_full = np.empty((514, N), dtype=ml_dtypes.bfloat16)
    lhsT_full[:512] = e1_bf.T
    lhsT_full[512:] = np.float32(1.0)

    iota = ((np.arange(512, dtype=np.float32) / 512.0)[None, :]
            .repeat(128, axis=0).astype(np.float16))

    tri = [l * 16 + m for l in range(15) for m in range(l + 1, 15)]
    tri += [255] * 7
    tri = np.array(tri, dtype=np.int16)                 # 112 entries
    triu = np.zeros((128, 7), dtype=np.int16)
    for p in range(128):
        for s in range(7):
            triu[p, s] = tri[s * 16 + (p & 15)]

    shared = dict(
        rhsT=rhsT, egather=e1_bf, iota=iota, triu=triu,
    )
    per_core = []
    for c in range(NCORES):
        lo = c * SHARD
        sl = slice(lo, lo + SHARD)
        bias = (MAGIC + (C0 - n1[sl].astype(np.float64)) * QSCALE).astype(np.float32)
        bias_t = bias.reshape(RT, 128).T.copy()         # [128, RT]
        selfidx = (np.arange(lo, lo + SHARD, dtype=np.float32)
                   .reshape(RT, 128).T.copy())
        refc_c = np.full((SHARD, 16), PAD_CURV, dtype=np.float32)
        refc_c[:, 0:15] = refc[sl, ::-1]
        refa_c = np.full((SHARD, 112), PAD_ANG, dtype=np.float32)
        refa_c[:, 0:105] = refa[sl, ::-1]
        per_core.append(dict(
            shared,
            lhsT=np.ascontiguousarray(lhsT_full[:, sl]),
            bias=bias_t,
            selfidx=selfidx,
            refc=refc_c,
            refa=refa_c,
        ))
    return per_core


_NC_CACHE = {}


def run_cores(inputs, debug_out=False, **run_kwargs):
    key = debug_out
    if key not in _NC_CACHE:
        _NC_CACHE[key] = build_nc(debug_out=debug_out)
    nc = _NC_CACHE[key]
    in_maps = _prep_inputs(**inputs)
    res = run_bass_kernel_spmd(
        nc, in_maps, core_ids=list(range(NCORES)), **run_kwargs
    )
    return res


def kernel(embeddings, reference_curvature_sig, reference_angular_sig):
    res = run_cores(dict(
        embeddings=embeddings,
        reference_curvature_sig=reference_curvature_sig,
        reference_angular_sig=reference_angular_sig,
    ))
    css = 0.0
    ass = 0.0
    for r in res.results:
        css += float(r["partial"][0, 0])
        ass += float(r["partial"][0, 1])
    curv_loss = css / (N * 15)
    ang_loss = ass / (N * 105)
    out = np.float32(0.3 * curv_loss + 0.7 * ang_loss)
    return np.asarray(out, dtype=np.float32)

